# revision 7
# baseline (speedup 1.0000x reference)
"""InterpretableMultiHeadAttention Trainium2 kernel (8 NeuronCores).

Sharding: core c -> (batch b = c//2, head-group g = c%2, 8 heads each).
Per-core device program (all matmuls bf16, accumulation fp32):
  phase 0: Q^T = WqblkT.T @ queryT (transposed projections), same K^T;
           V natural + ones column -> V1.
  phase 1 (per 256-wide q-block):
    S^T[k,q] per head via lhsT=K^T tile (contraction dh=64, 2-head row
    packing), exp on ACT (scale=1/8) -> E^T bf16,
    PV: lhsT=[V|1] accumulates ctx^T (rows 0..63) + softmax denom D (row 64),
    r = 1/D, broadcast via DRAM-bounce DMA,
    ctx-norm on DVE, out-proj on PE,
    avg^T += E^T * r per head (DVE mult, GpSimd accumulate).
Host: transposes/casts inputs, sums core pairs, adds biases, transposes avg.
"""

import os
import sys

for _p in ("/opt/trn_rl_repo", "/root/.axon_site/_ro/trn_rl_repo"):
    if os.path.isdir(_p) and _p not in sys.path:
        sys.path.insert(0, _p)

import numpy as np
import ml_dtypes

import concourse.bass as bass
import concourse.bacc as bacc
from concourse import mybir
from concourse.bass_utils import run_bass_kernel_spmd
from concourse.bass_interp import get_hw_module
from concourse.tile import TileContext

F32 = mybir.dt.float32
BF16 = mybir.dt.bfloat16
AF = mybir.ActivationFunctionType
OP = mybir.AluOpType
BF = ml_dtypes.bfloat16

B, S, D = 4, 2048, 1024
H, DH = 16, 64
NCORES = 8
HPC = 8            # heads per core
CS = 512           # channels per core (HPC * DH)
QB = 256           # q-block width
NQB = S // QB      # 8
MT = D // 128      # 8 m-tiles
ST = S // 128      # 16 s(k)-tiles
CT = CS // 128     # 4 c-tiles

_PROGRAM = None
LAST_RESULTS = None


def _build_program():
    nc = bacc.Bacc("TRN2", target_bir_lowering=False, debug=False,
                   num_devices=NCORES)

    qT_d = nc.dram_tensor("qT", [D, S], BF16, kind="ExternalInput")
    kT_d = nc.dram_tensor("kT", [D, S], BF16, kind="ExternalInput")
    vT_d = nc.dram_tensor("vT", [D, S], BF16, kind="ExternalInput")
    wqT_d = nc.dram_tensor("wqT", [D, CS], BF16, kind="ExternalInput")
    wkT_d = nc.dram_tensor("wkT", [D, CS], BF16, kind="ExternalInput")
    wvT_d = nc.dram_tensor("wvT", [D, CS], BF16, kind="ExternalInput")
    woT_d = nc.dram_tensor("woT", [CS, D], BF16, kind="ExternalInput")
    bq_d = nc.dram_tensor("bq", [128, CT], F32, kind="ExternalInput")
    bk_d = nc.dram_tensor("bk", [128, CT], F32, kind="ExternalInput")
    outp_d = nc.dram_tensor("outp", [S, D], F32, kind="ExternalOutput")
    avgT_d = nc.dram_tensor("avgT", [S, S], F32, kind="ExternalOutput")

    with TileContext(nc) as tc:
        with tc.tile_pool(name="persist", bufs=1) as persist, \
             tc.tile_pool(name="psc", bufs=3, space="PSUM") as psc, \
             tc.tile_pool(name="ppv", bufs=2, space="PSUM") as ppv, \
             tc.tile_pool(name="dscr", bufs=16, space="DRAM") as dscr, \
             tc.tile_pool(name="small", bufs=6) as small:

            # ---- persistent SBUF ----
            QT = [persist.tile([128, S], BF16, name=f"QT{i}") for i in range(CT)]
            KT = [persist.tile([128, S], BF16, name=f"KT{i}") for i in range(CT)]
            V1 = [persist.tile([128, HPC, DH + 1], BF16, name=f"V1_{i}")
                  for i in range(ST)]
            WO = [persist.tile([128, D], BF16, name=f"WO{i}") for i in range(CT)]
            CTX = [persist.tile([128, S], BF16, name=f"CTX{i}") for i in range(CT)]
            bq_sb = persist.tile([128, CT], F32, name="bq")
            bk_sb = persist.tile([128, CT], F32, name="bk")
            nc.sync.dma_start(out=bq_sb, in_=bq_d[:, :])
            nc.sync.dma_start(out=bk_sb, in_=bk_d[:, :])
            for i in range(CT):
                nc.sync.dma_start(out=WO[i], in_=woT_d[i * 128:(i + 1) * 128, :])

            # ---- phase 0: projections ----
            with tc.tile_pool(name="pin", bufs=10) as pin, \
                 tc.tile_pool(name="pw", bufs=9) as pw:
                # Q^T then K^T: out[dh, s] accumulating over m
                for (xT_dram, wT_dram, OUT, bias_sb) in (
                        (qT_d, wqT_d, QT, bq_sb), (kT_d, wkT_d, KT, bk_sb)):
                    xt = []
                    wt = []
                    for mt in range(MT):
                        x = pin.tile([128, S], BF16, tag="inT")
                        nc.sync.dma_start(out=x, in_=xT_dram[mt * 128:(mt + 1) * 128, :])
                        xt.append(x)
                        w = pw.tile([128, CS], BF16, tag="wT")
                        nc.sync.dma_start(out=w, in_=wT_dram[mt * 128:(mt + 1) * 128, :])
                        wt.append(w)
                    for dht in range(CT):
                        for sb in range(S // 512):
                            ps = psc.tile([128, 1024], F32, tag="sc")
                            for mt in range(MT):
                                nc.tensor.matmul(
                                    ps[:, 0:512],
                                    lhsT=wt[mt][:, dht * 128:(dht + 1) * 128],
                                    rhs=xt[mt][:, sb * 512:(sb + 1) * 512],
                                    start=(mt == 0), stop=(mt == MT - 1))
                            nc.vector.tensor_scalar(
                                OUT[dht][:, sb * 512:(sb + 1) * 512],
                                ps[:, 0:512], bias_sb[:, dht:dht + 1], None,
                                OP.add)
                # V natural [s, dh] -> V1 (ones in column DH of each head slot)
                vt = []
                wv = []
                for mt in range(MT):
                    x = pin.tile([128, S], BF16, tag="inT")
                    nc.sync.dma_start(out=x, in_=vT_d[mt * 128:(mt + 1) * 128, :])
                    vt.append(x)
                    w = pw.tile([128, CS], BF16, tag="wT")
                    nc.sync.dma_start(out=w, in_=wvT_d[mt * 128:(mt + 1) * 128, :])
                    wv.append(w)
                for st in range(ST):
                    nc.vector.memset(V1[st], 1.0)
                    ps = psc.tile([128, 1024], F32, tag="sc")
                    for mt in range(MT):
                        nc.tensor.matmul(
                            ps[:, 0:512],
                            lhsT=vt[mt][:, st * 128:(st + 1) * 128],
                            rhs=wv[mt],
                            start=(mt == 0), stop=(mt == MT - 1))
                    nc.scalar.activation(
                        V1[st][:, :, 0:DH],
                        ps[:, 0:512].rearrange("p (h d) -> p h d", h=HPC),
                        AF.Copy)

            # ---- phase 1 ----
            _nqb = int(os.environ.get("KERNEL_NQB", NQB))
            _step = int(os.environ.get("KERNEL_STEP", 9))
            with tc.tile_pool(name="pet", bufs=2) as pet, \
                 tc.tile_pool(name="pavg", bufs=1) as pavg, \
                 tc.tile_pool(name="ptmp", bufs=2) as ptmp, \
                 tc.tile_pool(name="prb", bufs=4) as prb, \
                 tc.tile_pool(name="post", bufs=4) as post:
                for qb in range(_nqb):
                    q0 = qb * QB
                    # scores + exp, two 4-head groups
                    ET = [pet.tile([128, ST, 4 * QB], BF16, tag="et",
                                   name=f"et_{qb}_{grp}")
                          for grp in range(2)]
                    # head j (within group) -> psum slice SLC[j]; row-64
                    # heads sit in a different bank than row-0 heads, since
                    # row-group-concurrent matmuls must not share a PSUM bank.
                    for kt in range(ST):
                        for grp in range(2):
                            ps = psc.tile([128, 1024], F32, tag="sc")
                            for j, slc in ((0, 0), (1, 2), (2, 1), (3, 3)):
                                h = grp * 4 + j
                                row = (h % 2) * 64
                                nc.tensor.matmul(
                                    ps[:, slc * QB:(slc + 1) * QB],
                                    lhsT=KT[h // 2][row:row + 64,
                                                    kt * 128:(kt + 1) * 128],
                                    rhs=QT[h // 2][row:row + 64, q0:q0 + QB],
                                    start=True, stop=True)
                            nc.scalar.activation(ET[grp][:, kt, :], ps,
                                                 AF.Exp, scale=0.125)
                    if _step < 2:
                        dbg = post.tile([128, QB], F32, tag="ost", name=f"dbg{qb}")
                        nc.vector.tensor_copy(dbg, ET[0][:, 0, 0:QB])
                        nc.sync.dma_start(
                            out=avgT_d.rearrange("(t p) q -> p t q", p=128)[
                                :, 0, q0:q0 + QB],
                            in_=dbg)
                        continue
                    avg = pavg.tile([128, ST, QB], F32, tag="avg")
                    SLC = (0, 2, 1, 3)
                    for h in range(HPC):
                        grp, j = h // 4, SLC[h % 4]
                        pv = ppv.tile([65, QB], F32, tag="pv")
                        for kt in range(ST):
                            nc.tensor.matmul(
                                pv,
                                lhsT=V1[kt][:, h, :],
                                rhs=ET[grp][:, kt, j * QB:(j + 1) * QB],
                                start=(kt == 0), stop=(kt == ST - 1))
                        if _step < 3:
                            continue
                        # r = 1/D ; broadcast via DRAM bounce
                        rt = prb.tile([128, QB], F32, tag="rt")
                        nc.vector.reciprocal(rt[64:65, :], pv[64:65, :])
                        rdram = dscr.tile([1, QB], F32, tag="rd")
                        nc.sync.dma_start(out=rdram, in_=rt[64:65, :])
                        rb = prb.tile([128, QB], F32, tag="rb")
                        bc = bass.AP(tensor=rdram.tensor, offset=rdram.offset,
                                     ap=[[0, 128]] + list(rdram.ap[1:]))
                        nc.sync.dma_start(out=rb, in_=bc)
                        rb16 = prb.tile([128, QB], BF16, tag="rb16")
                        nc.vector.tensor_copy(rb16, rb)
                        if _step < 4:
                            continue
                        # ctx-norm
                        if h % 2 == 0:
                            nc.vector.tensor_tensor(
                                CTX[h // 2][0:64, q0:q0 + QB],
                                pv[0:64, :], rb[0:64, :], OP.mult)
                        else:
                            cst = post.tile([64, QB], BF16, tag="cst")
                            nc.vector.tensor_tensor(
                                cst, pv[0:64, :], rb[0:64, :], OP.mult)
                            nc.sync.dma_start(
                                out=CTX[h // 2][64:128, q0:q0 + QB], in_=cst)
                        # avg accumulation
                        if _step < 5:
                            continue
                        rbb = bass.AP(tensor=rb16.tensor, offset=rb16.offset,
                                      ap=[rb16.ap[0], [0, ST], rb16.ap[1]])
                        eslc = ET[grp][:, :, j * QB:(j + 1) * QB]
                        if h == 0:
                            nc.vector.tensor_tensor(avg, eslc, rbb, OP.mult)
                        else:
                            tmp = ptmp.tile([128, ST, QB], BF16, tag="tmp")
                            nc.vector.tensor_tensor(tmp, eslc, rbb, OP.mult)
                            nc.gpsimd.tensor_tensor(avg, avg, tmp, OP.add)
                    if _step >= 5:
                        nc.sync.dma_start(
                            out=avgT_d.rearrange("(t p) q -> p t q", p=128)[
                                :, :, q0:q0 + QB],
                            in_=avg)
                    if _step < 6:
                        continue
                    # out-proj for this q-block
                    for qt2 in range(QB // 128):
                        qq = q0 + qt2 * 128
                        for eb in range(D // 512):
                            po = psc.tile([128, 1024], F32, tag="sc")
                            for ct in range(CT):
                                nc.tensor.matmul(
                                    po[:, 0:512],
                                    lhsT=CTX[ct][:, qq:qq + 128],
                                    rhs=WO[ct][:, eb * 512:(eb + 1) * 512],
                                    start=(ct == 0), stop=(ct == CT - 1))
                            ost = post.tile([128, 512], F32, tag="ost")
                            nc.vector.tensor_copy(ost, po[:, 0:512])
                            nc.sync.dma_start(
                                out=outp_d[qq:qq + 128,
                                           eb * 512:(eb + 1) * 512],
                                in_=ost)

    nc.compile()
    nc.m = get_hw_module(nc.m)
    return nc


def _get_program():
    global _PROGRAM
    if _PROGRAM is None:
        _PROGRAM = _build_program()
    return _PROGRAM


def kernel(query, key, value, Wq, bq, Wk, bk, Wv, bv, Wo, bo):
    global LAST_RESULTS
    query = np.asarray(query)
    key = np.asarray(key)
    value = np.asarray(value)
    Wq, bq = np.asarray(Wq), np.asarray(bq)
    Wk, bk = np.asarray(Wk), np.asarray(bk)
    Wv, bv = np.asarray(Wv), np.asarray(bv)
    Wo, bo = np.asarray(Wo), np.asarray(bo)

    nc = _get_program()

    in_maps = []
    for c in range(NCORES):
        b, g = c // 2, c % 2
        cs = g * CS
        in_maps.append({
            "qT": np.ascontiguousarray(query[b].T).astype(BF),
            "kT": np.ascontiguousarray(key[b].T).astype(BF),
            "vT": np.ascontiguousarray(value[b].T).astype(BF),
            "wqT": np.ascontiguousarray(Wq[cs:cs + CS, :].T).astype(BF),
            "wkT": np.ascontiguousarray(Wk[cs:cs + CS, :].T).astype(BF),
            "wvT": np.ascontiguousarray(Wv[cs:cs + CS, :].T).astype(BF),
            "woT": np.ascontiguousarray(Wo[:, cs:cs + CS].T).astype(BF),
            "bq": np.ascontiguousarray(
                bq[cs:cs + CS].reshape(CT, 128).T).astype(np.float32),
            "bk": np.ascontiguousarray(
                bk[cs:cs + CS].reshape(CT, 128).T).astype(np.float32),
        })

    trace = bool(os.environ.get("KERNEL_TRACE"))
    kw = {}
    if trace:
        kw = dict(trace=True, trace_cores=list(range(NCORES)),
                  stitch_traces=False)
    res = run_bass_kernel_spmd(nc, in_maps, core_ids=list(range(NCORES)), **kw)
    LAST_RESULTS = res

    out = np.empty((B, S, D), np.float32)
    avg = np.empty((B, S, S), np.float32)
    extra = (bv.astype(np.float32) @ Wo.astype(np.float32).T
             + bo.astype(np.float32))
    for b in range(B):
        r0, r1 = res.results[2 * b], res.results[2 * b + 1]
        out[b] = r0["outp"] + r1["outp"] + extra
        avg[b] = (r0["avgT"] + r1["avgT"]).T * (1.0 / H)
    return out, avg
